# revision 45
# baseline (speedup 1.0000x reference)
"""GQA attention (RoPE, causal) on 8 Trainium2 NeuronCores, tensor-parallel
over heads: each core owns 4 query heads + 1 kv head, computes its slice of
qkv, attention, and a partial output projection; the host sums the 8 partial
projections.

Single fused pipeline (no phase barriers): the qkv matmuls + RoPE of chunk
c+1 and the output projection of chunk c-1 are interleaved as background PE
units into the flash loop of chunk c, so the PE never idles long enough for
the HAM clock gate to drop it to half speed. All matmul dataflow is bf16
(fp32 PSUM). Scores are computed transposed ([st, sq]) so the softmax
denominator comes out of the attn@V matmul itself via a ones-column appended
to V (M=65), and exp needs no max-subtraction (logits are bounded; fp32 PSUM
can't overflow). ScalarE runs exp nearly exclusively; PSUM evacuation is
split between DVE and ScalarE; V transposes ride the DMA xbar.
"""

import numpy as np

HIDDEN = 2048
HEAD_DIM = 64
N_HEADS = 32
N_KV_HEADS = 8
S = 2048
N_CORES = 8
HPC = N_HEADS // N_CORES          # q heads per core = 4
D = HEAD_DIM
KT = HIDDEN // 128                # 16 contraction tiles for qkv
ST = S // 128                     # 16 seq tiles of 128
NC4 = S // 512                    # 4 seq chunks of 512
OSH = HPC * D + 2 * D             # 384 rows in the per-core qkv weight shard

_CACHE = {}


def _split_excess_waits(nc, mybir):
    """The staged walrus accepts at most one sync wait per instruction (two
    on EventSemaphore); Tile attaches more. Hoist extras onto same-engine
    NoOps inserted just before the instruction — engine program order then
    preserves the wait semantics."""
    for func in nc.m.functions:
        for block in func.blocks:
            new_insts = []
            for inst in block.instructions:
                si = inst.sync_info
                waits = list(si.on_wait) if si is not None and si.on_wait else []
                cap = 2 if isinstance(inst, mybir.InstEventSemaphore) else 1
                if len(waits) > cap:
                    si.on_wait = waits[:cap]
                    for j, w in enumerate(waits[cap:]):
                        nop = mybir.InstNoOp(
                            name=f"{inst.name}-ws{j}",
                            ins=[], outs=[], engine=inst.engine,
                        )
                        nop.sync_info = mybir.SyncInfo(on_wait=[w], on_update=[])
                        new_insts.append(nop)
                new_insts.append(inst)
            block.instructions = new_insts


def _build():
    import concourse.bass as bass
    import concourse.tile as tile
    from concourse import mybir

    f32 = mybir.dt.float32
    f32r = mybir.dt.float32r
    bf16 = mybir.dt.bfloat16

    nc = bass.Bass("TRN2", target_bir_lowering=False, debug=False,
                   num_devices=N_CORES)

    wo_d = nc.dram_tensor("woutT", [2 * 128, HIDDEN], bf16, kind="ExternalInput")
    c_d = nc.dram_tensor("ctile", [128, S], bf16, kind="ExternalInput")
    s_d = nc.dram_tensor("stile", [128, S], bf16, kind="ExternalInput")
    rt_d = nc.dram_tensor("rotT", [128, 128], bf16, kind="ExternalInput")
    eb_d = nc.dram_tensor("ebc", [128, 256], f32r, kind="ExternalInput")
    id_d = nc.dram_tensor("ident", [D, D], f32r, kind="ExternalInput")
    mk_d = nc.dram_tensor("masks", [4, 128, 512], bf16, kind="ExternalInput")
    out_d = nc.dram_tensor("out", [HIDDEN, S], bf16, kind="ExternalOutput")

    # host ships x and wq pre-swizzled to [128, k, ...] so the whole stream
    # moves in four big DMAs (HWDGE triggers cost ~0.65us each)
    xP_d = nc.dram_tensor("xP", [128, KT * S], bf16, kind="ExternalInput")
    wqP_d = nc.dram_tensor("wqP", [128, KT * OSH], bf16, kind="ExternalInput")
    xP_t = xP_d.rearrange("p (t c s) -> p t c s", t=KT, c=2)
    wqP_t = wqP_d.rearrange("p (t o) -> p t o", t=KT)
    outR = out_d.rearrange("(a p) s -> p a s", p=128)

    scale = 1.0 / float(np.sqrt(D))
    Exp = mybir.ActivationFunctionType.Exp

    with tile.TileContext(nc) as tc:
        with (
            nc.allow_low_precision(reason="bf16 dataflow is deliberate"),
            tc.tile_pool(name="wts", bufs=1) as wts,
            tc.tile_pool(name="acts", bufs=1) as acts,
            tc.tile_pool(name="psb", bufs=4) as psb,
            tc.tile_pool(name="ev", bufs=2) as evp,
            tc.tile_pool(name="evo", bufs=4) as evo,
            tc.tile_pool(name="qkvp", bufs=1, space="PSUM") as qkvp,
            tc.tile_pool(name="scp", bufs=2, space="PSUM") as scp,
            tc.tile_pool(name="avp", bufs=2, space="PSUM") as avp,
            tc.tile_pool(name="wkp", bufs=1, space="PSUM") as wkp,
        ):
            # ---- persistent tiles ----
            wq_all = wts.tile([128, KT, OSH], bf16, tag="wqa", name="wqa")
            x_all = wts.tile([128, KT, 2, 1024], bf16, tag="xa", name="xa")
            ct = wts.tile([128, S], bf16, tag="ct", name="ct")
            st = wts.tile([128, S], bf16, tag="st", name="st")
            rt = wts.tile([128, 128], bf16, tag="rt", name="rt")
            eb = wts.tile([128, 256], f32r, tag="eb", name="eb")
            idt = wts.tile([D, D], f32r, tag="idt", name="idt")
            mk = [wts.tile([128, 512], bf16, tag=f"mk{j}", name="mk")
                  for j in range(4)]
            wo = [wts.tile([128, HIDDEN], bf16, tag=f"wo{i}", name="wo")
                  for i in range(2)]

            qr = [acts.tile([128, S], bf16, tag=f"qr{p}", name=f"qr{p}")
                  for p in range(2)]
            # k duplicated on both partition halves so score matmuls can
            # slice the half matching qr's base partition (PE requires
            # lhsT/rhs base partitions to agree)
            kro = acts.tile([128, S], bf16, tag="kro", name="kro")
            vT = acts.tile([D, S], f32r, tag="vT", name="vT")
            v_sb = acts.tile([128, ST, D + 1], bf16, tag="v", name="v")
            outT = [acts.tile([128, S], bf16, tag=f"oT{p}", name=f"oT{p}")
                    for p in range(2)]

            # ---- prologue DMAs. x/wq arrive pre-swizzled in four big
            # transfers split across the sync and scalar HWDGE queues (the
            # scalar queue is idle until the first evac). The gpsimd SWDGE
            # queue only carries tiny consts — its descriptor generation
            # contends with DVE's SBUF port. ----
            nc.gpsimd.dma_start(rt[:], rt_d[:])
            nc.gpsimd.dma_start(eb[:], eb_d[:])
            nc.gpsimd.dma_start(idt[:], id_d[:])
            nc.gpsimd.memset(v_sb[:, :, D:D + 1], 1.0)
            for j in range(4):
                nc.gpsimd.dma_start(mk[j][:], mk_d[j])

            # PE warmup: dummy matmuls on the tiny early-arriving rot matrix
            # keep the HAM activity window busy while x/wq stream in, so the
            # first real matmuls run at 2.4GHz instead of the cold 1.2GHz
            for w in range(40):
                wu = wkp.tile([128, 512], f32, tag="wk", name="wu")
                nc.tensor.matmul(wu[:, 0:128], lhsT=rt[:], rhs=rt[:],
                                 start=True, stop=True)

            KH = KT // 2
            # chunk-0's columns land first in fine slices so qkv(c0) starts
            # as early as possible; later chunks ride big batched transfers
            nc.sync.dma_start(wq_all[:, 0:KH, :], wqP_t[:, 0:KH, :])
            nc.scalar.dma_start(wq_all[:, KH:KT, :], wqP_t[:, KH:KT, :])
            nc.sync.dma_start(x_all[:, 0:KH, 0, 0:512],
                              xP_t[:, 0:KH, 0, 0:512])
            nc.scalar.dma_start(x_all[:, KH:KT, 0, 0:512],
                                xP_t[:, KH:KT, 0, 0:512])
            nc.sync.dma_start(x_all[:, 0:KH, 0, 512:1024],
                              xP_t[:, 0:KH, 0, 512:1024])
            nc.scalar.dma_start(x_all[:, KH:KT, 0, 512:1024],
                                xP_t[:, KH:KT, 0, 512:1024])
            nc.sync.dma_start(ct[:], c_d[:])
            nc.scalar.dma_start(st[:], s_d[:])
            nc.sync.dma_start(x_all[:, 0:KH, 1, :], xP_t[:, 0:KH, 1, :])
            nc.scalar.dma_start(x_all[:, KH:KT, 1, :], xP_t[:, KH:KT, 1, :])
            for i in range(2):
                nc.sync.dma_start(wo[i][:], wo_d[i * 128:(i + 1) * 128, :])

            # ---- background-unit builders (each unit = a small batch of
            # instructions; drained between flash groups so the PE stream
            # stays dense) ----
            def qkv_units(c):
                csl = slice(c * 512, (c + 1) * 512)
                ps_h = {}
                units = []

                xsl = slice((c % 2) * 512, (c % 2) * 512 + 512)

                def mk_pass(o, k0, k1):
                    def u():
                        if k0 == 0:
                            ps_h[o] = qkvp.tile([128, 512], f32, tag="qkv",
                                                name="qkv")
                        ps = ps_h[o]
                        for k in range(k0, k1):
                            nc.tensor.matmul(
                                ps[:],
                                lhsT=wq_all[:, k, o * 128:(o + 1) * 128],
                                rhs=x_all[:, k, c // 2, xsl],
                                start=(k == 0), stop=(k == KT - 1))
                    return u

                def mk_rope_q(o):
                    def u():
                        ps = ps_h[o]
                        nc.scalar.copy(qr[o][:, csl], ps[:])
                        sw = wkp.tile([128, 512], f32, tag="wk", name="sw")
                        nc.tensor.matmul(sw[:], lhsT=rt[:], rhs=qr[o][:, csl],
                                         start=True, stop=True)
                        m1 = evp.tile([128, 512], bf16, tag="m1", name="m1")
                        nc.vector.tensor_mul(m1[:], qr[o][:, csl], ct[:, csl])
                        m2 = evp.tile([128, 512], bf16, tag="m2", name="m2")
                        nc.vector.tensor_mul(m2[:], sw[:], st[:, csl])
                        nc.vector.tensor_add(qr[o][:, csl], m1[:], m2[:])
                    return u

                def mk_rope_kv():
                    def u():
                        ps = ps_h[2]
                        nc.scalar.copy(kro[0:D, csl], ps[0:D, :])
                        nc.scalar.copy(kro[D:128, csl], ps[0:D, :])
                        nc.scalar.copy(vT[:, csl], ps[D:128, :])
                        sw = wkp.tile([128, 512], f32, tag="wk", name="sw")
                        nc.tensor.matmul(sw[:], lhsT=rt[:],
                                         rhs=kro[:, csl], start=True, stop=True)
                        m1 = evp.tile([128, 512], bf16, tag="m1", name="m1")
                        nc.vector.tensor_mul(m1[:], kro[:, csl], ct[:, csl])
                        m2 = evp.tile([128, 512], bf16, tag="m2", name="m2")
                        nc.vector.tensor_mul(m2[:], sw[:], st[:, csl])
                        nc.vector.tensor_add(kro[:, csl], m1[:], m2[:])
                    return u

                def mk_vtrans():
                    def u():
                        for t in range(4 * c, 4 * c + 4):
                            pv = wkp.tile([128, 512], f32r, tag="wk",
                                          name="pv")
                            nc.tensor.transpose(
                                pv[:, 0:D], vT[:, t * 128:(t + 1) * 128],
                                idt[:])
                            nc.vector.tensor_copy(v_sb[:, t, 0:D],
                                                  pv[:, 0:D])
                    return u

                for o in range(3):
                    units.append(mk_pass(o, 0, 8))
                    units.append(mk_pass(o, 8, KT))
                    units.append(mk_rope_q(o) if o < 2 else mk_rope_kv())
                units.append(mk_vtrans())
                return units

            uo_tiles = {}
            rcr_tiles = {}

            def norm_unit(c):
                csl = slice(c * 512, (c + 1) * 512)

                def u():
                    rcr = rcr_tiles.pop(c)
                    for pair in range(2):
                        bc = wkp.tile([128, 512], f32, tag="wk", name="bc")
                        nc.tensor.matmul(
                            bc[:], lhsT=eb[0:97, pair * 128:(pair + 1) * 128],
                            rhs=rcr[0:97, :], start=True, stop=True)
                        uo = uo_tiles.pop((c, pair))
                        nc.vector.tensor_mul(outT[pair][:, csl], uo[:], bc[:])
                return u

            ev4_h = {}

            def proj_unit(c, ht):
                csl = slice(c * 512, (c + 1) * 512)

                def u():
                    # the last chunk's proj runs in the drained pipeline tail:
                    # borrow the freed score-PSUM banks and the exp-free
                    # ScalarE so the evac chain pipelines across engines
                    tail = c == NC4 - 1
                    pool = scp if tail else wkp
                    tag = "sc" if tail else "wk"
                    pr = pool.tile([128, 512], f32, tag=tag, name="pr")
                    for i in range(2):
                        nc.tensor.matmul(
                            pr[:],
                            lhsT=wo[i][:, ht * 128:(ht + 1) * 128],
                            rhs=outT[i][:, csl],
                            start=(i == 0), stop=(i == 1))
                    if ht % 4 == 0:
                        ev4_h[c] = evo.tile([128, 4, 512], bf16, tag="ev",
                                            name="ev")
                    ev4 = ev4_h[c]
                    if tail and ht % 2 == 1:
                        nc.scalar.copy(ev4[:, ht % 4, :], pr[:])
                    else:
                        nc.vector.tensor_copy(ev4[:, ht % 4, :], pr[:])
                    if ht % 4 == 3:
                        # one batched store per 4 row-blocks (fewer triggers)
                        nc.sync.dma_start(outR[:, ht - 3:ht + 1, csl],
                                          ev4[:, :, :])
                return u

            def drain(bg, n):
                for _ in range(min(n, len(bg))):
                    bg.pop(0)()

            # ---- flash attention for one chunk, draining background units
            # between groups ----
            def flash_chunk(c, bg, per_group):
                csl = slice(c * 512, (c + 1) * 512)
                n_st = 4 * c + 4
                # denominators land on rows 32h (ACT writes must start on a
                # 32-aligned partition); the in-between rows are memset to 1
                # so their reciprocals stay finite (eb zeros them out)
                l_sb = evp.tile([128, 512], f32, tag="l", name="l")
                nc.gpsimd.memset(l_sb[0:97, :], 1.0)
                for h in range(HPC):
                    pair, half = divmod(h, 2)
                    if half == 0:
                        uo_tiles[(c, pair)] = evp.tile(
                            [128, 512], f32, tag=f"uo{pair}", name=f"uo{pair}")
                    uo = uo_tiles[(c, pair)]
                    av = avp.tile([128, 512], f32, tag="av", name="av")

                    def av_group(g, pt):
                        for i in range(2):
                            t = 2 * g + i
                            nc.tensor.matmul(
                                av[0:D + 1, :],
                                lhsT=v_sb[:, t, :],
                                rhs=pt[:, i * 512:(i + 1) * 512],
                                start=(t == 0), stop=(t == n_st - 1))

                    prev = None
                    for g in range(n_st // 2):
                        sc = scp.tile([128, 1024], f32, tag="sc", name="sc")
                        for i in range(2):
                            t = 2 * g + i
                            nc.tensor.matmul(
                                sc[:, i * 512:(i + 1) * 512],
                                lhsT=kro[half * D:(half + 1) * D,
                                         t * 128:(t + 1) * 128],
                                rhs=qr[pair][half * D:(half + 1) * D, csl],
                                start=True, stop=True)
                        pt = psb.tile([128, 1024], bf16, tag="P", name="P")
                        nc.scalar.activation(pt[:], sc[:], Exp, scale=scale)
                        for i in range(2):
                            t = 2 * g + i
                            if t >= 4 * c:
                                nc.vector.tensor_mul(
                                    pt[:, i * 512:(i + 1) * 512],
                                    pt[:, i * 512:(i + 1) * 512],
                                    mk[t - 4 * c][:])
                        if prev is not None:
                            av_group(*prev)
                        drain(bg, per_group)
                        prev = (g, pt)
                    av_group(*prev)
                    nc.vector.tensor_copy(uo[half * D:(half + 1) * D, :],
                                          av[0:D, :])
                    nc.scalar.copy(l_sb[32 * h:32 * h + 1, :], av[D:D + 1, :])
                    if c == NC4 - 1 and half == 1:
                        # last chunk: normalize per pair as soon as its two
                        # heads finish, so the tail chain overlaps the
                        # remaining flash work
                        psl = slice(64 * pair, 64 * pair + 33)
                        rcp = evp.tile([128, 512], f32, tag="rcp", name="rcp")
                        nc.vector.reciprocal(rcp[psl, :], l_sb[psl, :])
                        rcr = evp.tile([128, 512], f32r, tag="rcr", name="rcr")
                        nc.vector.tensor_copy(rcr[psl, :], rcp[psl, :])
                        bc = wkp.tile([128, 512], f32, tag="wk", name="bc")
                        nc.tensor.matmul(
                            bc[:], lhsT=eb[psl, pair * 128:(pair + 1) * 128],
                            rhs=rcr[psl, :], start=True, stop=True)
                        nc.vector.tensor_mul(outT[pair][:, csl],
                                             uo_tiles.pop((c, pair))[:],
                                             bc[:])
                if c < NC4 - 1:
                    rcp = evp.tile([128, 512], f32, tag="rcp", name="rcp")
                    nc.vector.reciprocal(rcp[0:97, :], l_sb[0:97, :])
                    rcr = evp.tile([128, 512], f32r, tag="rcr", name="rcr")
                    nc.vector.tensor_copy(rcr[0:97, :], rcp[0:97, :])
                    rcr_tiles[c] = rcr

            # ---- the pipeline ----
            for u in qkv_units(0):
                u()

            def interleave(a, b):
                out, ia, ib = [], 0, 0
                while ia < len(a) or ib < len(b):
                    if ia < len(a):
                        out.append(a[ia]); ia += 1
                    if ib < len(b):
                        out.append(b[ib]); ib += 1
                return out

            for c in range(NC4):
                post = [proj_unit(c - 1, ht) for ht in range(16)] if c > 0 else []
                pre = qkv_units(c + 1) if c + 1 < NC4 else []
                # norm before any proj (emission order defines dataflow), but
                # after two qkv units so the bc matmul never head-of-line
                # blocks the PE behind the DVE reciprocal
                head, rest = pre[:4], pre[4:]
                norm = [norm_unit(c - 1)] if c > 0 else []
                bg = head + norm + interleave(rest, post)
                # pre-drain so the PE has work while the previous chunk's
                # rope/epilogue chain finishes on ScalarE/DVE (norm stays a
                # few groups in, behind the DVE reciprocal)
                drain(bg, 2)
                flash_chunk(c, bg, 2 if c < 3 else 1)
                drain(bg, len(bg))

            for ht in range(16):
                proj_unit(NC4 - 1, ht)()

    _split_excess_waits(nc, mybir)
    return nc


def _host_prep(x, cos, sin, w_qkv, w_out):
    import ml_dtypes
    bf = ml_dtypes.bfloat16

    xT = np.ascontiguousarray(x[0].T).astype(bf)                # [H, S]
    # [128, kt*2*1024]: partition-major swizzle so x moves in 2 big DMAs
    xP = np.ascontiguousarray(
        xT.reshape(KT, 128, 2, 1024).transpose(1, 0, 2, 3).reshape(128, -1))
    cosT = cos.T.astype(np.float32)                             # [64, S]
    sinT = sin.T.astype(np.float32)
    ctile = np.ascontiguousarray(np.concatenate([cosT, cosT], 0)).astype(bf)
    stile = np.ascontiguousarray(np.concatenate([sinT, sinT], 0)).astype(bf)

    # rotate_half as a matrix: rot(q)^T = R @ q^T per 64-block; ship R^T
    r = np.zeros((D, D), dtype=np.float32)
    for i in range(32):
        r[i, 32 + i] = -1.0
        r[32 + i, i] = 1.0
    R = np.zeros((128, 128), dtype=np.float32)
    R[:D, :D] = r
    R[D:, D:] = r
    rotT = np.ascontiguousarray(R.T).astype(bf)

    # ebc[32h, 128*pair + j] = 1 iff h == 2*pair + j//64  (head-broadcast of
    # the reciprocal rows at 32h onto the stacked [128, 512] norm tiles)
    ebc = np.zeros((128, 256), dtype=np.float32)
    for pair in range(2):
        for j in range(128):
            ebc[32 * (2 * pair + j // 64), 128 * pair + j] = 1.0

    p = np.arange(128)[:, None]
    f = np.arange(512)[None, :]
    masks = np.stack([(p <= f - 128 * j).astype(bf) for j in range(4)])

    shared = {"xP": xP, "ctile": ctile, "stile": stile, "rotT": rotT,
              "ebc": ebc, "masks": masks,
              "ident": np.eye(D, dtype=np.float32)}

    in_maps = []
    for c in range(N_CORES):
        qrows = w_qkv[4 * c * D:(4 * c + 4) * D]                # [256, H]
        krows = w_qkv[N_HEADS * D + c * D: N_HEADS * D + (c + 1) * D]
        vrows = w_qkv[(N_HEADS + N_KV_HEADS) * D + c * D:
                      (N_HEADS + N_KV_HEADS) * D + (c + 1) * D]
        wsh = np.concatenate([qrows, krows, vrows], 0)          # [384, H]
        wqkvT = np.ascontiguousarray(wsh.T).astype(bf)          # [H, 384]
        wqP = np.ascontiguousarray(
            wqkvT.reshape(KT, 128, OSH).transpose(1, 0, 2).reshape(128, -1))
        wo_cols = w_out[:, 4 * c * D:(4 * c + 4) * D]           # [H, 256]
        woutT = np.ascontiguousarray(wo_cols.T).astype(bf)
        in_maps.append({**shared, "wqP": wqP, "woutT": woutT})
    return in_maps


def kernel(x, cos, sin, w_qkv, w_out):
    from concourse.bass_utils import run_bass_kernel_spmd

    if "nc" not in _CACHE:
        _CACHE["nc"] = _build()
    nc = _CACHE["nc"]

    in_maps = _host_prep(x, cos, sin, w_qkv, w_out)
    res = run_bass_kernel_spmd(nc, in_maps, list(range(N_CORES)))
    total = np.zeros((HIDDEN, S), dtype=np.float32)
    for r in res.results:
        total += r["out"].astype(np.float32)
    return total.T.reshape(1, S, HIDDEN).copy()


# revision 48
# speedup vs baseline: 1.0681x; 1.0681x over previous
"""GQA attention (RoPE, causal) on 8 Trainium2 NeuronCores, tensor-parallel
over heads: each core owns 4 query heads + 1 kv head, computes its slice of
qkv, attention, and a partial output projection; the host sums the 8 partial
projections.

Single fused pipeline (no phase barriers): the qkv matmuls + RoPE of chunk
c+1 and the output projection of chunk c-1 are interleaved as background PE
units into the flash loop of chunk c, so the PE never idles long enough for
the HAM clock gate to drop it to half speed. All matmul dataflow is bf16
(fp32 PSUM). Scores are computed transposed ([st, sq]) so the softmax
denominator comes out of the attn@V matmul itself via a ones-column appended
to V (M=65), and exp needs no max-subtraction (logits are bounded; fp32 PSUM
can't overflow). ScalarE runs exp nearly exclusively; PSUM evacuation is
split between DVE and ScalarE; V transposes ride the DMA xbar.
"""

import numpy as np

HIDDEN = 2048
HEAD_DIM = 64
N_HEADS = 32
N_KV_HEADS = 8
S = 2048
N_CORES = 8
HPC = N_HEADS // N_CORES          # q heads per core = 4
D = HEAD_DIM
KT = HIDDEN // 128                # 16 contraction tiles for qkv
ST = S // 128                     # 16 seq tiles of 128
NC4 = S // 512                    # 4 seq chunks of 512
OSH = HPC * D + 2 * D             # 384 rows in the per-core qkv weight shard

_CACHE = {}


def _split_excess_waits(nc, mybir):
    """The staged walrus accepts at most one sync wait per instruction (two
    on EventSemaphore); Tile attaches more. Hoist extras onto same-engine
    NoOps inserted just before the instruction — engine program order then
    preserves the wait semantics."""
    for func in nc.m.functions:
        for block in func.blocks:
            new_insts = []
            for inst in block.instructions:
                si = inst.sync_info
                waits = list(si.on_wait) if si is not None and si.on_wait else []
                cap = 2 if isinstance(inst, mybir.InstEventSemaphore) else 1
                if len(waits) > cap:
                    si.on_wait = waits[:cap]
                    for j, w in enumerate(waits[cap:]):
                        nop = mybir.InstNoOp(
                            name=f"{inst.name}-ws{j}",
                            ins=[], outs=[], engine=inst.engine,
                        )
                        nop.sync_info = mybir.SyncInfo(on_wait=[w], on_update=[])
                        new_insts.append(nop)
                new_insts.append(inst)
            block.instructions = new_insts


def _build():
    import concourse.bass as bass
    import concourse.tile as tile
    from concourse import mybir

    f32 = mybir.dt.float32
    f32r = mybir.dt.float32r
    bf16 = mybir.dt.bfloat16

    nc = bass.Bass("TRN2", target_bir_lowering=False, debug=False,
                   num_devices=N_CORES)

    wo_d = nc.dram_tensor("woutT", [2 * 128, HIDDEN], bf16, kind="ExternalInput")
    c_d = nc.dram_tensor("ctile", [128, S], bf16, kind="ExternalInput")
    s_d = nc.dram_tensor("stile", [128, S], bf16, kind="ExternalInput")
    rt_d = nc.dram_tensor("rotT", [128, 128], bf16, kind="ExternalInput")
    eb_d = nc.dram_tensor("ebc", [128, 256], f32r, kind="ExternalInput")
    id_d = nc.dram_tensor("ident", [D, D], f32r, kind="ExternalInput")
    mk_d = nc.dram_tensor("masks", [4, 128, 512], bf16, kind="ExternalInput")
    out_d = nc.dram_tensor("out", [HIDDEN, S], bf16, kind="ExternalOutput")

    # host ships x and wq pre-swizzled to [128, k, ...] so the whole stream
    # moves in four big DMAs (HWDGE triggers cost ~0.65us each)
    xP_d = nc.dram_tensor("xP", [128, KT * S], bf16, kind="ExternalInput")
    wqP_d = nc.dram_tensor("wqP", [128, KT * OSH], bf16, kind="ExternalInput")
    xP_t = xP_d.rearrange("p (t c s) -> p t c s", t=KT, c=2)
    wqP_t = wqP_d.rearrange("p (t o) -> p t o", t=KT)
    outR = out_d.rearrange("(a p) s -> p a s", p=128)

    scale = 1.0 / float(np.sqrt(D))
    Exp = mybir.ActivationFunctionType.Exp
    Ln = mybir.ActivationFunctionType.Ln

    with tile.TileContext(nc) as tc:
        with (
            nc.allow_low_precision(reason="bf16 dataflow is deliberate"),
            tc.tile_pool(name="wts", bufs=1) as wts,
            tc.tile_pool(name="acts", bufs=1) as acts,
            tc.tile_pool(name="psb", bufs=4) as psb,
            tc.tile_pool(name="ev", bufs=2) as evp,
            tc.tile_pool(name="evo", bufs=4) as evo,
            tc.tile_pool(name="qkvp", bufs=1, space="PSUM") as qkvp,
            tc.tile_pool(name="scp", bufs=2, space="PSUM") as scp,
            tc.tile_pool(name="avp", bufs=2, space="PSUM") as avp,
            tc.tile_pool(name="wkp", bufs=1, space="PSUM") as wkp,
        ):
            # ---- persistent tiles ----
            wq_all = wts.tile([128, KT, OSH], bf16, tag="wqa", name="wqa")
            x_all = wts.tile([128, KT, 2, 1024], bf16, tag="xa", name="xa")
            ct = wts.tile([128, S], bf16, tag="ct", name="ct")
            st = wts.tile([128, S], bf16, tag="st", name="st")
            rt = wts.tile([128, 128], bf16, tag="rt", name="rt")
            eb = wts.tile([128, 256], f32r, tag="eb", name="eb")
            idt = wts.tile([D, D], f32r, tag="idt", name="idt")
            mk = [wts.tile([128, 512], bf16, tag=f"mk{j}", name="mk")
                  for j in range(4)]
            wo = [wts.tile([128, HIDDEN], bf16, tag=f"wo{i}", name="wo")
                  for i in range(2)]

            qr = [acts.tile([128, S], bf16, tag=f"qr{p}", name=f"qr{p}")
                  for p in range(2)]
            # k duplicated on both partition halves so score matmuls can
            # slice the half matching qr's base partition (PE requires
            # lhsT/rhs base partitions to agree)
            kro = acts.tile([128, S], bf16, tag="kro", name="kro")
            vT = acts.tile([D, S], f32r, tag="vT", name="vT")
            v_sb = acts.tile([128, ST, D + 1], bf16, tag="v", name="v")
            outT = [acts.tile([128, S], bf16, tag=f"oT{p}", name=f"oT{p}")
                    for p in range(2)]

            # ---- prologue DMAs. x/wq arrive pre-swizzled in four big
            # transfers split across the sync and scalar HWDGE queues (the
            # scalar queue is idle until the first evac). The gpsimd SWDGE
            # queue only carries tiny consts — its descriptor generation
            # contends with DVE's SBUF port. ----
            nc.sync.dma_start(rt[:], rt_d[:])
            nc.gpsimd.dma_start(eb[:], eb_d[:])
            nc.gpsimd.dma_start(idt[:], id_d[:])
            nc.gpsimd.memset(v_sb[:, :, D:D + 1], 1.0)
            for j in range(4):
                nc.gpsimd.dma_start(mk[j][:], mk_d[j])

            # PE warmup: dummy matmuls on the tiny early-arriving rot matrix
            # keep the HAM activity window busy while x/wq stream in, so the
            # first real matmuls run at 2.4GHz instead of the cold 1.2GHz
            for w in range(40):
                wu = wkp.tile([128, 512], f32, tag="wk", name="wu")
                nc.tensor.matmul(wu[:, 0:128], lhsT=rt[:], rhs=rt[:],
                                 start=True, stop=True)

            KH = KT // 2
            # chunk-0's columns land first in fine slices so qkv(c0) starts
            # as early as possible; later chunks ride big batched transfers
            nc.sync.dma_start(wq_all[:, 0:KH, :], wqP_t[:, 0:KH, :])
            nc.scalar.dma_start(wq_all[:, KH:KT, :], wqP_t[:, KH:KT, :])
            nc.sync.dma_start(x_all[:, 0:KH, 0, 0:512],
                              xP_t[:, 0:KH, 0, 0:512])
            nc.scalar.dma_start(x_all[:, KH:KT, 0, 0:512],
                                xP_t[:, KH:KT, 0, 0:512])
            nc.sync.dma_start(x_all[:, 0:KH, 0, 512:1024],
                              xP_t[:, 0:KH, 0, 512:1024])
            nc.scalar.dma_start(x_all[:, KH:KT, 0, 512:1024],
                                xP_t[:, KH:KT, 0, 512:1024])
            nc.sync.dma_start(ct[:], c_d[:])
            nc.scalar.dma_start(st[:], s_d[:])
            nc.sync.dma_start(x_all[:, 0:KH, 1, :], xP_t[:, 0:KH, 1, :])
            nc.scalar.dma_start(x_all[:, KH:KT, 1, :], xP_t[:, KH:KT, 1, :])
            for i in range(2):
                nc.sync.dma_start(wo[i][:], wo_d[i * 128:(i + 1) * 128, :])

            # ---- background-unit builders (each unit = a small batch of
            # instructions; drained between flash groups so the PE stream
            # stays dense) ----
            def qkv_units(c):
                csl = slice(c * 512, (c + 1) * 512)
                ps_h = {}
                units = []

                xsl = slice((c % 2) * 512, (c % 2) * 512 + 512)

                def mk_pass(o, k0, k1):
                    def u():
                        if k0 == 0:
                            ps_h[o] = qkvp.tile([128, 512], f32, tag="qkv",
                                                name="qkv")
                        ps = ps_h[o]
                        for k in range(k0, k1):
                            nc.tensor.matmul(
                                ps[:],
                                lhsT=wq_all[:, k, o * 128:(o + 1) * 128],
                                rhs=x_all[:, k, c // 2, xsl],
                                start=(k == 0), stop=(k == KT - 1))
                    return u

                def mk_rope_q(o):
                    def u():
                        ps = ps_h[o]
                        nc.scalar.copy(qr[o][:, csl], ps[:])
                        sw = wkp.tile([128, 512], f32, tag="wk", name="sw")
                        nc.tensor.matmul(sw[:], lhsT=rt[:], rhs=qr[o][:, csl],
                                         start=True, stop=True)
                        m1 = evp.tile([128, 512], bf16, tag="m1", name="m1")
                        nc.vector.tensor_mul(m1[:], qr[o][:, csl], ct[:, csl])
                        m2 = evp.tile([128, 512], bf16, tag="m2", name="m2")
                        nc.vector.tensor_mul(m2[:], sw[:], st[:, csl])
                        nc.vector.tensor_add(qr[o][:, csl], m1[:], m2[:])
                    return u

                def mk_rope_kv():
                    def u():
                        ps = ps_h[2]
                        nc.scalar.copy(kro[0:D, csl], ps[0:D, :])
                        nc.scalar.copy(kro[D:128, csl], ps[0:D, :])
                        nc.scalar.copy(vT[:, csl], ps[D:128, :])
                        sw = wkp.tile([128, 512], f32, tag="wk", name="sw")
                        nc.tensor.matmul(sw[:], lhsT=rt[:],
                                         rhs=kro[:, csl], start=True, stop=True)
                        m1 = evp.tile([128, 512], bf16, tag="m1", name="m1")
                        nc.vector.tensor_mul(m1[:], kro[:, csl], ct[:, csl])
                        m2 = evp.tile([128, 512], bf16, tag="m2", name="m2")
                        nc.vector.tensor_mul(m2[:], sw[:], st[:, csl])
                        nc.vector.tensor_add(kro[:, csl], m1[:], m2[:])
                    return u

                def mk_vtrans():
                    def u():
                        for t in range(4 * c, 4 * c + 4):
                            pv = wkp.tile([128, 512], f32r, tag="wk",
                                          name="pv")
                            nc.tensor.transpose(
                                pv[:, 0:D], vT[:, t * 128:(t + 1) * 128],
                                idt[:])
                            nc.vector.tensor_copy(v_sb[:, t, 0:D],
                                                  pv[:, 0:D])
                    return u

                for o in range(3):
                    units.append(mk_pass(o, 0, 8))
                    units.append(mk_pass(o, 8, KT))
                    units.append(mk_rope_q(o) if o < 2 else mk_rope_kv())
                units.append(mk_vtrans())
                return units

            uo_tiles = {}
            rcr_tiles = {}

            def norm_unit(c):
                csl = slice(c * 512, (c + 1) * 512)

                def u():
                    rcr = rcr_tiles.pop(c)
                    for pair in range(2):
                        bc = wkp.tile([128, 512], f32, tag="wk", name="bc")
                        nc.tensor.matmul(
                            bc[:], lhsT=eb[0:97, pair * 128:(pair + 1) * 128],
                            rhs=rcr[0:97, :], start=True, stop=True)
                        uo = uo_tiles.pop((c, pair))
                        nc.vector.tensor_mul(outT[pair][:, csl], uo[:], bc[:])
                return u

            ev4_h = {}

            def proj_unit(c, ht):
                csl = slice(c * 512, (c + 1) * 512)

                def u():
                    # the last chunk's proj runs in the drained pipeline tail:
                    # borrow the freed score-PSUM banks and the exp-free
                    # ScalarE so the evac chain pipelines across engines
                    tail = c == NC4 - 1
                    pool = scp if tail else wkp
                    tag = "sc" if tail else "wk"
                    pr = pool.tile([128, 512], f32, tag=tag, name="pr")
                    for i in range(2):
                        nc.tensor.matmul(
                            pr[:],
                            lhsT=wo[i][:, ht * 128:(ht + 1) * 128],
                            rhs=outT[i][:, csl],
                            start=(i == 0), stop=(i == 1))
                    if ht % 4 == 0:
                        ev4_h[c] = evo.tile([128, 4, 512], bf16, tag="ev",
                                            name="ev")
                    ev4 = ev4_h[c]
                    if tail and ht % 2 == 1:
                        nc.scalar.copy(ev4[:, ht % 4, :], pr[:])
                    else:
                        nc.vector.tensor_copy(ev4[:, ht % 4, :], pr[:])
                    if ht % 4 == 3:
                        # one batched store per 4 row-blocks (fewer triggers)
                        nc.sync.dma_start(outR[:, ht - 3:ht + 1, csl],
                                          ev4[:, :, :])
                return u

            def drain(bg, n):
                for _ in range(min(n, len(bg))):
                    bg.pop(0)()

            # ---- flash attention for one chunk, draining background units
            # between groups ----
            def flash_chunk(c, bg, per_group):
                csl = slice(c * 512, (c + 1) * 512)
                n_st = 4 * c + 4
                # denominators land on rows 32h (ACT writes must start on a
                # 32-aligned partition); the in-between rows are memset to 1
                # so their reciprocals stay finite (eb zeros them out)
                l_sb = evp.tile([128, 512], f32, tag="l", name="l")
                nc.gpsimd.memset(l_sb[0:97, :], 1.0)
                for h in range(HPC):
                    pair, half = divmod(h, 2)
                    if half == 0:
                        uo_tiles[(c, pair)] = evp.tile(
                            [128, 512], f32, tag=f"uo{pair}", name=f"uo{pair}")
                    uo = uo_tiles[(c, pair)]
                    av = avp.tile([128, 512], f32, tag="av", name="av")

                    def av_group(g, pt):
                        for i in range(2):
                            t = 2 * g + i
                            nc.tensor.matmul(
                                av[0:D + 1, :],
                                lhsT=v_sb[:, t, :],
                                rhs=pt[:, i * 512:(i + 1) * 512],
                                start=(t == 0), stop=(t == n_st - 1))

                    prev = None
                    for g in range(n_st // 2):
                        sc = scp.tile([128, 1024], f32, tag="sc", name="sc")
                        for i in range(2):
                            t = 2 * g + i
                            nc.tensor.matmul(
                                sc[:, i * 512:(i + 1) * 512],
                                lhsT=kro[half * D:(half + 1) * D,
                                         t * 128:(t + 1) * 128],
                                rhs=qr[pair][half * D:(half + 1) * D, csl],
                                start=True, stop=True)
                        pt = psb.tile([128, 1024], bf16, tag="P", name="P")
                        nc.scalar.activation(pt[:], sc[:], Exp, scale=scale)
                        for i in range(2):
                            t = 2 * g + i
                            if t >= 4 * c:
                                nc.vector.tensor_mul(
                                    pt[:, i * 512:(i + 1) * 512],
                                    pt[:, i * 512:(i + 1) * 512],
                                    mk[t - 4 * c][:])
                        if prev is not None:
                            av_group(*prev)
                        drain(bg, per_group)
                        prev = (g, pt)
                    av_group(*prev)
                    nc.vector.tensor_copy(uo[half * D:(half + 1) * D, :],
                                          av[0:D, :])
                    nc.scalar.copy(l_sb[32 * h:32 * h + 1, :], av[D:D + 1, :])
                    if c == NC4 - 1 and half == 1:
                        # last chunk: normalize per pair as soon as its two
                        # heads finish, so the tail chain overlaps the
                        # remaining flash work. 1/l = exp(-ln l) keeps the
                        # whole chain on the already-hot ScalarE (walrus
                        # loads the combined natural_log_exp table set).
                        psl = slice(64 * pair, 64 * pair + 33)
                        rcl = evp.tile([128, 512], f32, tag="rcl", name="rcl")
                        nc.scalar.activation(rcl[psl, :], l_sb[psl, :], Ln)
                        rcp = evp.tile([128, 512], f32, tag="rcp", name="rcp")
                        nc.scalar.activation(rcp[psl, :], rcl[psl, :], Exp,
                                             scale=-1.0)
                        rcr = evp.tile([128, 512], f32r, tag="rcr", name="rcr")
                        nc.vector.tensor_copy(rcr[psl, :], rcp[psl, :])
                        bc = scp.tile([128, 512], f32, tag="sc", name="bc")
                        nc.tensor.matmul(
                            bc[:], lhsT=eb[psl, pair * 128:(pair + 1) * 128],
                            rhs=rcr[psl, :], start=True, stop=True)
                        nc.vector.tensor_mul(outT[pair][:, csl],
                                             uo_tiles.pop((c, pair))[:],
                                             bc[:])
                if c < NC4 - 1:
                    rcl = evp.tile([128, 512], f32, tag="rcl", name="rcl")
                    nc.scalar.activation(rcl[0:97, :], l_sb[0:97, :], Ln)
                    rcp = evp.tile([128, 512], f32, tag="rcp", name="rcp")
                    nc.scalar.activation(rcp[0:97, :], rcl[0:97, :], Exp,
                                         scale=-1.0)
                    rcr = evp.tile([128, 512], f32r, tag="rcr", name="rcr")
                    nc.vector.tensor_copy(rcr[0:97, :], rcp[0:97, :])
                    rcr_tiles[c] = rcr

            # ---- the pipeline ----
            for u in qkv_units(0):
                u()

            def interleave(a, b):
                out, ia, ib = [], 0, 0
                while ia < len(a) or ib < len(b):
                    if ia < len(a):
                        out.append(a[ia]); ia += 1
                    if ib < len(b):
                        out.append(b[ib]); ib += 1
                return out

            for c in range(NC4):
                post = [proj_unit(c - 1, ht) for ht in range(16)] if c > 0 else []
                pre = qkv_units(c + 1) if c + 1 < NC4 else []
                # norm before any proj (emission order defines dataflow), but
                # after two qkv units so the bc matmul never head-of-line
                # blocks the PE behind the DVE reciprocal
                head, rest = pre[:4], pre[4:]
                norm = [norm_unit(c - 1)] if c > 0 else []
                bg = head + norm + interleave(rest, post)
                # pre-drain so the PE has work while the previous chunk's
                # rope/epilogue chain finishes on ScalarE/DVE (norm stays a
                # few groups in, behind the DVE reciprocal)
                drain(bg, 2)
                flash_chunk(c, bg, 2 if c < 3 else 1)
                drain(bg, len(bg))

            for ht in range(16):
                proj_unit(NC4 - 1, ht)()

    _split_excess_waits(nc, mybir)
    return nc


def _host_prep(x, cos, sin, w_qkv, w_out):
    import ml_dtypes
    bf = ml_dtypes.bfloat16

    xT = np.ascontiguousarray(x[0].T).astype(bf)                # [H, S]
    # [128, kt*2*1024]: partition-major swizzle so x moves in 2 big DMAs
    xP = np.ascontiguousarray(
        xT.reshape(KT, 128, 2, 1024).transpose(1, 0, 2, 3).reshape(128, -1))
    cosT = cos.T.astype(np.float32)                             # [64, S]
    sinT = sin.T.astype(np.float32)
    ctile = np.ascontiguousarray(np.concatenate([cosT, cosT], 0)).astype(bf)
    stile = np.ascontiguousarray(np.concatenate([sinT, sinT], 0)).astype(bf)

    # rotate_half as a matrix: rot(q)^T = R @ q^T per 64-block; ship R^T
    r = np.zeros((D, D), dtype=np.float32)
    for i in range(32):
        r[i, 32 + i] = -1.0
        r[32 + i, i] = 1.0
    R = np.zeros((128, 128), dtype=np.float32)
    R[:D, :D] = r
    R[D:, D:] = r
    rotT = np.ascontiguousarray(R.T).astype(bf)

    # ebc[32h, 128*pair + j] = 1 iff h == 2*pair + j//64  (head-broadcast of
    # the reciprocal rows at 32h onto the stacked [128, 512] norm tiles)
    ebc = np.zeros((128, 256), dtype=np.float32)
    for pair in range(2):
        for j in range(128):
            ebc[32 * (2 * pair + j // 64), 128 * pair + j] = 1.0

    p = np.arange(128)[:, None]
    f = np.arange(512)[None, :]
    masks = np.stack([(p <= f - 128 * j).astype(bf) for j in range(4)])

    shared = {"xP": xP, "ctile": ctile, "stile": stile, "rotT": rotT,
              "ebc": ebc, "masks": masks,
              "ident": np.eye(D, dtype=np.float32)}

    in_maps = []
    for c in range(N_CORES):
        qrows = w_qkv[4 * c * D:(4 * c + 4) * D]                # [256, H]
        krows = w_qkv[N_HEADS * D + c * D: N_HEADS * D + (c + 1) * D]
        vrows = w_qkv[(N_HEADS + N_KV_HEADS) * D + c * D:
                      (N_HEADS + N_KV_HEADS) * D + (c + 1) * D]
        wsh = np.concatenate([qrows, krows, vrows], 0)          # [384, H]
        wqkvT = np.ascontiguousarray(wsh.T).astype(bf)          # [H, 384]
        wqP = np.ascontiguousarray(
            wqkvT.reshape(KT, 128, OSH).transpose(1, 0, 2).reshape(128, -1))
        wo_cols = w_out[:, 4 * c * D:(4 * c + 4) * D]           # [H, 256]
        woutT = np.ascontiguousarray(wo_cols.T).astype(bf)
        in_maps.append({**shared, "wqP": wqP, "woutT": woutT})
    return in_maps


def kernel(x, cos, sin, w_qkv, w_out):
    from concourse.bass_utils import run_bass_kernel_spmd

    if "nc" not in _CACHE:
        _CACHE["nc"] = _build()
    nc = _CACHE["nc"]

    in_maps = _host_prep(x, cos, sin, w_qkv, w_out)
    res = run_bass_kernel_spmd(nc, in_maps, list(range(N_CORES)))
    total = np.zeros((HIDDEN, S), dtype=np.float32)
    for r in res.results:
        total += r["out"].astype(np.float32)
    return total.T.reshape(1, S, HIDDEN).copy()
